# revision 51
# baseline (speedup 1.0000x reference)
"""Multi-head attention TRN2 kernel, head-sharded across 8 NeuronCores.

Problem: B=4, S=2048, D_IN=512, D_H=512, H=8.
Each core computes one head end-to-end:
    qT    = scaled Wq.T-projection of x (score scale folded into Wq,bq)
    kT, v = projections of the host-COMPACTED key sequence: masked keys have
            exactly zero attention weight, so the host gathers only valid
            key rows (padded to a multiple of 128) and the key-proportional
            work (kT, v, scores, AV) runs on ~56% of the sequence
    sT    = scores TRANSPOSED [t, s] so the pad-mask becomes a per-partition
            bias of the exp() activation
    pT    = exp(sT + maskoff)             (unnormalized probs, transposed)
    oT    = v.T @ pT; denom via DVE running sum + ones-row matmul
    part  = (oT.T @ Wp_h) * (1/denom)     (normalization deferred to the end)
Host: out = sum_h part_h + (bp + sum_h bv_h @ Wp_h)   (bv folded out since
      probs sum to 1). x / x_kv are passed pre-transposed (feature-major) so
      no on-device transposes are needed.

All matmuls run in float32r (1 cycle/row at N>=256; ~2e-4 rel err).
"""

import math
from contextlib import ExitStack
from functools import lru_cache

import numpy as np

import concourse.tile as tile
from concourse import bacc, mybir
from concourse.bass_utils import run_bass_kernel_spmd

B, S, D, H = 4, 2048, 512, 8
NCORES = 8
MASK_NEG = -30000.0

F32 = mybir.dt.float32
F32R = mybir.dt.float32r
AF = mybir.ActivationFunctionType


def _emit(nc, b_sz, s_sz, kv_tiles, rep=1):
    s_kv = max(kv_tiles) * 128   # padded DRAM width shared by all batches
    NSB = s_sz // 512      # query blocks
    NC = D // 128          # 128-chunks of the feature dim

    xt_d = nc.dram_tensor("xt", [b_sz, NC, 128, s_sz], F32, kind="ExternalInput")
    xkvt_d = nc.dram_tensor("xkvt", [b_sz, NC, 128, s_kv], F32, kind="ExternalInput")
    mo_d = nc.dram_tensor("maskoff", [b_sz, 128, s_kv // 128], F32, kind="ExternalInput")
    wq_d = nc.dram_tensor("wq", [D, D], F32, kind="ExternalInput")
    wk_d = nc.dram_tensor("wk", [D, D], F32, kind="ExternalInput")
    wv_d = nc.dram_tensor("wv", [D, D], F32, kind="ExternalInput")
    wp_d = nc.dram_tensor("wp", [D, D], F32, kind="ExternalInput")
    bq_d = nc.dram_tensor("bq", [128, NC], F32, kind="ExternalInput")
    bk_d = nc.dram_tensor("bk", [128, NC], F32, kind="ExternalInput")
    out_d = nc.dram_tensor("out", [b_sz, s_sz, D], F32, kind="ExternalOutput")
    # denominators of the very last query block (normalized on host so the
    # kernel tail doesn't wait on the reciprocal chain)
    dl_d = nc.dram_tensor("denlast", [1, 512], F32, kind="ExternalOutput")

    def make_groups(nt):
        # chunk the kv width into pieces of 256..512 (f32r needs N>=256 for
        # full rate); a lone 128 tail steals 256 from the previous chunk
        widths = []
        remt = nt
        while remt > 0:
            take = min(4, remt)
            widths.append(take)
            remt -= take
        if len(widths) > 1 and widths[-1] == 1:
            widths[-2] -= 2
            widths[-1] += 2
        groups = []
        pos = 0
        for w in widths:
            groups.append(list(range(pos, pos + w)))
            pos += w
        return groups

    with tile.TileContext(nc) as tc, ExitStack() as ctx:
        ep = ctx.enter_context
        cpool = ep(tc.tile_pool(name="const", bufs=1))
        wpool = ep(tc.tile_pool(name="w", bufs=1))
        mpool = ep(tc.tile_pool(name="mask", bufs=2))
        xtqp = ep(tc.tile_pool(name="xtq", bufs=1))
        xtkp = ep(tc.tile_pool(name="xtk", bufs=1))
        ktp = ep(tc.tile_pool(name="kt", bufs=1))
        vp = ep(tc.tile_pool(name="v", bufs=1))
        qtp = ep(tc.tile_pool(name="qt", bufs=2))
        ptp = ep(tc.tile_pool(name="pt", bufs=6))
        otp = ep(tc.tile_pool(name="ot", bufs=2))
        recp = ep(tc.tile_pool(name="rec", bufs=2))
        srp = ep(tc.tile_pool(name="sr", bufs=2))
        resp = ep(tc.tile_pool(name="res", bufs=3))
        drp = ep(tc.tile_pool(name="dr", bufs=2, space="DRAM"))
        pop = ep(tc.tile_pool(name="po", bufs=4, space="PSUM"))
        psp = ep(tc.tile_pool(name="pss", bufs=2, space="PSUM"))
        pbp = ep(tc.tile_pool(name="psb", bufs=2, space="PSUM"))

        ones_f = cpool.tile([128, 1], F32)
        nc.vector.memset(ones_f[:], 1.0)
        ones = cpool.tile([128, 1], F32R)
        nc.vector.tensor_copy(ones[:], ones_f[:])

        wq = wpool.tile([128, NC, D], F32R)
        wk = wpool.tile([128, NC, D], F32R)
        wv = wpool.tile([128, NC, D], F32R)
        wp = wpool.tile([128, NC, D], F32R)
        bq_t = wpool.tile([128, NC], F32)
        bk_t = wpool.tile([128, NC], F32)

        # weight loads staggered between the first batch's x loads so the PE
        # never waits long on DMA
        _wloads = {
            "wk0": lambda: nc.sync.dma_start(
                wk[:, :, 0:128],
                wk_d.ap()[:, 0:128].rearrange("(c p) e -> p c e", p=128).bitcast(F32R),
            ),
            "wk": lambda: (
                nc.sync.dma_start(bq_t[:], bq_d.ap()),
                nc.sync.dma_start(bk_t[:], bk_d.ap()),
                nc.sync.dma_start(
                    wk[:, :, 128:D],
                    wk_d.ap()[:, 128:D]
                    .rearrange("(c p) e -> p c e", p=128)
                    .bitcast(F32R),
                ),
            ),
            "wv": lambda: nc.sync.dma_start(
                wv[:], wv_d.ap().rearrange("(c p) e -> p c e", p=128).bitcast(F32R)
            ),
            "wq": lambda: nc.sync.dma_start(
                wq[:], wq_d.ap().rearrange("(c p) e -> p c e", p=128).bitcast(F32R)
            ),
            "wp": lambda: nc.sync.dma_start(
                wp[:], wp_d.ap().rearrange("(c p) e -> p c e", p=128).bitcast(F32R)
            ),
        }

        def load_weights(*names):
            for n in names:
                fn = _wloads.pop(n, None)
                if fn is not None:
                    fn()

        pending = None  # deferred final-projection stage (oT, rT, b, sb)

        def flush_pending():
            nonlocal pending
            if pending is None:
                return
            oT, rT, b, sb = pending
            for j in range(4):
                pf = pbp.tile([128, 512], F32, tag="pbig", name="pf")
                for m in range(NC):
                    nc.tensor.matmul(
                        pf[:],
                        oT[:, m, j * 128 : (j + 1) * 128],
                        wp[:, m, :],
                        start=(m == 0),
                        stop=(m == NC - 1),
                    )
                res = resp.tile([128, 512], F32)
                if rT is None:  # final block: host normalizes
                    nc.vector.tensor_copy(res[:], pf[:])
                else:
                    nc.vector.tensor_scalar_mul(res[:], pf[:], rT[:, j : j + 1])
                r0 = sb * 512 + j * 128
                nc.sync.dma_start(out_d.ap()[b, r0 : r0 + 128, :], res[:])
            pending = None

        batch_seq = [b for _ in range(rep) for b in range(b_sz)]
        for it, b in enumerate(batch_seq):
            is_last_batch = it == len(batch_seq) - 1
            nt_b = kv_tiles[b]
            kv_groups = make_groups(nt_b)
            mo_t = mpool.tile([128, nt_b], F32)
            nc.sync.dma_start(mo_t[:], mo_d.ap()[b][:, :nt_b])

            # ---- stage P: load xT slices, project k^T and v ----
            xTq = xtqp.tile([128, NC, s_sz], F32R)
            xTk = xtkp.tile([128, NC, nt_b * 128], F32R)
            kT = ktp.tile([128, NC, nt_b * 128], F32R)
            v = vp.tile([128, nt_b, D], F32R)
            for gi, tiles in enumerate(kv_groups):
                last = gi == len(kv_groups) - 1
                n0 = tiles[0] * 128
                nw = len(tiles) * 128
                load_weights("wk0")
                for c in range(NC):
                    nc.sync.dma_start(
                        xTk[:, c, n0 : n0 + nw],
                        xkvt_d.ap()[b, c, :, n0 : n0 + nw].bitcast(F32R),
                    )
                load_weights("wk")
                if gi >= 1 or last:
                    load_weights("wq")
                if gi >= 2 or last:
                    load_weights("wp")
                # c-major across 4 parallel banks (po pool is idle in stage P)
                # so the PE issues 4 matmuls per arriving xTk chunk
                pks = [
                    pop.tile([128, 512], F32, tag="po", name=f"psk{m}")
                    for m in range(NC)
                ]
                for c in range(NC):
                    for m in range(NC):
                        nc.tensor.matmul(
                            pks[m][:, :nw],
                            wk[:, c, m * 128 : (m + 1) * 128],
                            xTk[:, c, n0 : n0 + nw],
                            start=(c == 0),
                            stop=(c == NC - 1),
                        )
                for m in range(NC):
                    nc.scalar.activation(
                        kT[:, m, n0 : n0 + nw],
                        pks[m][:, :nw],
                        AF.Identity,
                        bias=bk_t[:, m : m + 1],
                    )
                load_weights("wv")
                for t in tiles:
                    ps = psp.tile([128, 512], F32, tag="psmall", name="psv")
                    for c in range(NC):
                        nc.tensor.matmul(
                            ps[:],
                            xTk[:, c, t * 128 : (t + 1) * 128],
                            wv[:, c, :],
                            start=(c == 0),
                            stop=(c == NC - 1),
                        )
                    nc.vector.tensor_copy(v[:, t, :], ps[:])
                # stream the q-side xT in alongside (needed from stage A on)
                if gi < NC:
                    nc.sync.dma_start(
                        xTq[:, gi, :], xt_d.ap()[b, gi, :, :].bitcast(F32R)
                    )
            for c in range(len(kv_groups), NC):
                nc.sync.dma_start(xTq[:, c, :], xt_d.ap()[b, c, :, :].bitcast(F32R))

            # ---- stage A: per query-block attention ----
            for sb in range(NSB):
                qT = qtp.tile([128, NC, 512], F32R)
                for m in range(NC):
                    ps = psp.tile([128, 512], F32, tag="psmall", name="psq")
                    for c in range(NC):
                        nc.tensor.matmul(
                            ps[:],
                            wq[:, c, m * 128 : (m + 1) * 128],
                            xTq[:, c, sb * 512 : (sb + 1) * 512],
                            start=(c == 0),
                            stop=(c == NC - 1),
                        )
                    nc.scalar.activation(
                        qT[:, m, :], ps[:], AF.Identity, bias=bq_t[:, m : m + 1]
                    )

                flush_pending()

                po = [
                    pop.tile([128, 512], F32, tag="po", name=f"po{i}")
                    for i in range(NC)
                ]
                srun = srp.tile([128, 512], F32)
                srun_r = None

                def av_group(t, ptile):
                    for m in range(NC):
                        nc.tensor.matmul(
                            po[m][:],
                            v[:, t, m * 128 : (m + 1) * 128],
                            ptile[:],
                            start=(t == 0),
                            stop=(t == nt_b - 1),
                        )

                prev_av = None
                for t in range(nt_b):
                    ps = psp.tile([128, 512], F32, tag="psmall", name="pss")
                    for c in range(NC):
                        nc.tensor.matmul(
                            ps[:],
                            kT[:, c, t * 128 : (t + 1) * 128],
                            qT[:, c, :],
                            start=(c == 0),
                            stop=(c == NC - 1),
                        )
                    if prev_av is not None:
                        av_group(*prev_av)
                    ptile = ptp.tile([128, 512], F32R)
                    nc.scalar.activation(
                        ptile[:], ps[:], AF.Exp, bias=mo_t[:, t : t + 1]
                    )
                    # running sum on the otherwise-idle GPSIMD engine so the
                    # DVE never delays ptile release (which gates ACT exp and
                    # in turn the PE scores pipeline)
                    if t < nt_b - 1:
                        if t == 0:
                            nc.gpsimd.tensor_copy(srun[:], ptile[:].bitcast(F32))
                        else:
                            nc.gpsimd.tensor_add(
                                srun[:], srun[:], ptile[:].bitcast(F32)
                            )
                    else:
                        # final step rounds to f32r so the partition-reduce
                        # matmul below runs at f32r speed
                        srun_r = srp.tile([128, 512], F32R, name="srun_r")
                        if t == 0:
                            nc.gpsimd.tensor_copy(srun_r[:], ptile[:])
                        else:
                            nc.gpsimd.tensor_add(
                                srun_r[:], srun[:], ptile[:].bitcast(F32)
                            )
                    prev_av = (t, ptile)
                av_group(*prev_av)
                # partition-reduce the running sum
                pd = pbp.tile([1, 512], F32, tag="pbig")
                nc.tensor.matmul(pd[:], ones[:], srun_r[:], start=True, stop=True)

                oT = otp.tile([128, NC, 512], F32R)
                for m in range(NC):
                    if m % 2 == 0:
                        nc.vector.tensor_copy(oT[:, m, :], po[m][:])
                    else:
                        nc.scalar.activation(oT[:, m, :], po[m][:], AF.Copy)
                if is_last_batch and sb == NSB - 1:
                    # final block: ship the denominators; host divides
                    den = recp.tile([1, 512], F32)
                    nc.vector.tensor_copy(den[:], pd[:])
                    nc.sync.dma_start(dl_d.ap(), den[:])
                    rT = None
                else:
                    den = recp.tile([1, 512], F32)
                    nc.vector.tensor_copy(den[:], pd[:])
                    dscr = drp.tile([1, 512], F32)
                    nc.sync.dma_start(dscr[:], den[:])
                    denT = recp.tile([128, 4], F32)
                    nc.sync.dma_start(
                        denT[:], dscr[0, :].rearrange("(j p) -> p j", p=128)
                    )
                    rT = recp.tile([128, 4], F32)
                    nc.vector.reciprocal(rT[:], denT[:])

                pending = (oT, rT, b, sb)

        flush_pending()


@lru_cache(maxsize=4)
def _build(b_sz, s_sz, kv_tiles, rep=1):
    nc = bacc.Bacc("TRN2", target_bir_lowering=False, debug=False)
    _emit(nc, b_sz, s_sz, kv_tiles, rep=rep)
    nc.compile()
    return nc


def _prep_inputs(x, mask, Wq, bq, Wk, bk, Wv, bv, Wp, bp):
    """Host-side shard prep. Returns (in_maps, bp_eff, kv_tiles)."""
    b_sz, s_sz, _ = x.shape
    nc_ = D // 128
    x = np.asarray(x, dtype=np.float32)
    m = np.asarray(mask).reshape(b_sz, s_sz)
    counts = (m != 0).sum(axis=1)
    kv_tiles = tuple(max(1, int(-(-int(c) // 128))) for c in counts)
    s_kv = max(kv_tiles) * 128
    nt_kv = s_kv // 128
    # compact the key sequence: gather valid rows, pad with zeros + mask
    x_kv = np.zeros((b_sz, s_kv, D), dtype=np.float32)
    moff = np.full((b_sz, s_kv), np.float32(MASK_NEG), dtype=np.float32)
    for b in range(b_sz):
        idx = np.nonzero(m[b])[0]
        x_kv[b, : len(idx)] = x[b, idx]
        moff[b, : len(idx)] = 0.0
    moff = np.ascontiguousarray(moff.reshape(b_sz, nt_kv, 128).transpose(0, 2, 1))
    # feature-major (transposed) copies: [b, c, p, s]
    xt = np.ascontiguousarray(
        x.transpose(0, 2, 1).reshape(b_sz, nc_, 128, s_sz)
    )
    xkvt = np.ascontiguousarray(
        x_kv.transpose(0, 2, 1).reshape(b_sz, nc_, 128, s_kv)
    )

    sc = 1.0 / math.sqrt(D)
    in_maps = []
    for h in range(NCORES):
        wq_h = np.ascontiguousarray(np.asarray(Wq[h], dtype=np.float32) * sc)
        bq_h = (np.asarray(bq[h], dtype=np.float32) * sc).reshape(4, 128).T
        bk_h = np.asarray(bk[h], dtype=np.float32).reshape(4, 128).T
        in_maps.append(
            {
                "xt": xt,
                "xkvt": xkvt,
                "maskoff": moff,
                "wq": wq_h,
                "wk": np.ascontiguousarray(np.asarray(Wk[h], dtype=np.float32)),
                "wv": np.ascontiguousarray(np.asarray(Wv[h], dtype=np.float32)),
                "wp": np.ascontiguousarray(
                    np.asarray(Wp[h * D : (h + 1) * D, :], dtype=np.float32)
                ),
                "bq": np.ascontiguousarray(bq_h),
                "bk": np.ascontiguousarray(bk_h),
            }
        )
    bv64 = np.asarray(bv, dtype=np.float64)
    wp64 = np.asarray(Wp, dtype=np.float64)
    bp_eff = np.asarray(bp, dtype=np.float64).copy()
    for h in range(NCORES):
        bp_eff += bv64[h] @ wp64[h * D : (h + 1) * D, :]
    return in_maps, bp_eff.astype(np.float32), kv_tiles


def combine_results(results, bp_eff, b_sz, s_sz):
    """Sum per-head partials; the final query block arrives unnormalized with
    its denominators in 'denlast'."""
    acc = np.zeros((b_sz, s_sz, D), dtype=np.float64)
    for h in range(NCORES):
        o = np.asarray(results[h]["out"], dtype=np.float64)
        den = np.asarray(results[h]["denlast"], dtype=np.float64).reshape(512)
        acc[: b_sz - 1] += o[: b_sz - 1]
        acc[b_sz - 1, : s_sz - 512] += o[b_sz - 1, : s_sz - 512]
        acc[b_sz - 1, s_sz - 512 :] += o[b_sz - 1, s_sz - 512 :] / den[:, None]
    acc += bp_eff
    return acc.astype(np.float32)


def kernel(x, mask, Wq, bq, Wk, bk, Wv, bv, Wp, bp):
    x = np.asarray(x)
    b_sz, s_sz, _ = x.shape
    in_maps, bp_eff, kv_tiles = _prep_inputs(x, mask, Wq, bq, Wk, bk, Wv, bv, Wp, bp)
    nc = _build(b_sz, s_sz, kv_tiles)
    res = run_bass_kernel_spmd(nc, in_maps, list(range(NCORES)))
    return combine_results(res.results, bp_eff, b_sz, s_sz)


# revision 54
# speedup vs baseline: 1.0792x; 1.0792x over previous
"""Multi-head attention TRN2 kernel, head-sharded across 8 NeuronCores.

Problem: B=4, S=2048, D_IN=512, D_H=512, H=8.
Each core computes one head end-to-end:
    qT    = scaled Wq.T-projection of x (score scale folded into Wq,bq)
    kT, v = projections of the host-COMPACTED key sequence: masked keys have
            exactly zero attention weight, so the host gathers only valid
            key rows (padded to a multiple of 128) and the key-proportional
            work (kT, v, scores, AV) runs on ~56% of the sequence
    sT    = scores TRANSPOSED [t, s] so the pad-mask becomes a per-partition
            bias of the exp() activation
    pT    = exp(sT + maskoff)             (unnormalized probs, transposed)
    oT    = v.T @ pT; denom via DVE running sum + ones-row matmul
    part  = (oT.T @ Wp_h) * (1/denom)     (normalization deferred to the end)
Host: out = sum_h part_h + (bp + sum_h bv_h @ Wp_h)   (bv folded out since
      probs sum to 1). x / x_kv are passed pre-transposed (feature-major) so
      no on-device transposes are needed.

All matmuls run in float32r (1 cycle/row at N>=256; ~2e-4 rel err).
"""

import math
from contextlib import ExitStack
from functools import lru_cache

import numpy as np

import concourse.tile as tile
from concourse import bacc, mybir
from concourse.bass_utils import run_bass_kernel_spmd

B, S, D, H = 4, 2048, 512, 8
NCORES = 8
MASK_NEG = -30000.0

F32 = mybir.dt.float32
F32R = mybir.dt.float32r
AF = mybir.ActivationFunctionType


def _emit(nc, b_sz, s_sz, kv_tiles, rep=1):
    s_kv = max(kv_tiles) * 128   # padded DRAM width shared by all batches
    NSB = s_sz // 512      # query blocks
    NC = D // 128          # 128-chunks of the feature dim

    xt_d = nc.dram_tensor("xt", [b_sz, NC, 128, s_sz], F32, kind="ExternalInput")
    xkvt_d = nc.dram_tensor("xkvt", [b_sz, NC, 128, s_kv], F32, kind="ExternalInput")
    mo_d = nc.dram_tensor("maskoff", [b_sz, 128, s_kv // 128], F32, kind="ExternalInput")
    wq_d = nc.dram_tensor("wq", [D, D], F32, kind="ExternalInput")
    wk_d = nc.dram_tensor("wk", [D, D], F32, kind="ExternalInput")
    wv_d = nc.dram_tensor("wv", [D, D], F32, kind="ExternalInput")
    wp_d = nc.dram_tensor("wp", [D, D], F32, kind="ExternalInput")
    bq_d = nc.dram_tensor("bq", [128, NC], F32, kind="ExternalInput")
    bk_d = nc.dram_tensor("bk", [128, NC], F32, kind="ExternalInput")
    out_d = nc.dram_tensor("out", [b_sz, s_sz, D], F32, kind="ExternalOutput")
    # denominators of the very last query block (normalized on host so the
    # kernel tail doesn't wait on the reciprocal chain)
    dl_d = nc.dram_tensor("denlast", [1, 512], F32, kind="ExternalOutput")

    def make_groups(nt):
        # chunk the kv width into pieces of 256..512 (f32r needs N>=256 for
        # full rate); a lone 128 tail steals 256 from the previous chunk
        widths = []
        remt = nt
        while remt > 0:
            take = min(4, remt)
            widths.append(take)
            remt -= take
        if len(widths) > 1 and widths[-1] == 1:
            widths[-2] -= 2
            widths[-1] += 2
        groups = []
        pos = 0
        for w in widths:
            groups.append(list(range(pos, pos + w)))
            pos += w
        return groups

    with tile.TileContext(nc) as tc, ExitStack() as ctx:
        ep = ctx.enter_context
        cpool = ep(tc.tile_pool(name="const", bufs=1))
        wpool = ep(tc.tile_pool(name="w", bufs=1))
        mpool = ep(tc.tile_pool(name="mask", bufs=2))
        xtqp = ep(tc.tile_pool(name="xtq", bufs=1))
        xtkp = ep(tc.tile_pool(name="xtk", bufs=1))
        ktp = ep(tc.tile_pool(name="kt", bufs=1))
        vp = ep(tc.tile_pool(name="v", bufs=1))
        qtp = ep(tc.tile_pool(name="qt", bufs=2))
        ptp = ep(tc.tile_pool(name="pt", bufs=4))
        otp = ep(tc.tile_pool(name="ot", bufs=2))
        recp = ep(tc.tile_pool(name="rec", bufs=2))
        srp = ep(tc.tile_pool(name="sr", bufs=2))
        resp = ep(tc.tile_pool(name="res", bufs=3))
        drp = ep(tc.tile_pool(name="dr", bufs=2, space="DRAM"))
        pop = ep(tc.tile_pool(name="po", bufs=4, space="PSUM"))
        psp = ep(tc.tile_pool(name="pss", bufs=2, space="PSUM"))
        pbp = ep(tc.tile_pool(name="psb", bufs=2, space="PSUM"))

        ones_f = cpool.tile([128, 1], F32)
        nc.vector.memset(ones_f[:], 1.0)
        ones = cpool.tile([128, 1], F32R)
        nc.vector.tensor_copy(ones[:], ones_f[:])

        wq = wpool.tile([128, NC, D], F32R)
        wk = wpool.tile([128, NC, D], F32R)
        wv = wpool.tile([128, NC, D], F32R)
        wp = wpool.tile([128, NC, D], F32R)
        bq_t = wpool.tile([128, NC], F32)
        bk_t = wpool.tile([128, NC], F32)

        # weight loads staggered between the first batch's x loads so the PE
        # never waits long on DMA
        _wloads = {
            "wk0": lambda: nc.sync.dma_start(
                wk[:, :, 0:128],
                wk_d.ap()[:, 0:128].rearrange("(c p) e -> p c e", p=128).bitcast(F32R),
            ),
            "wk": lambda: (
                nc.sync.dma_start(bq_t[:], bq_d.ap()),
                nc.sync.dma_start(bk_t[:], bk_d.ap()),
                nc.sync.dma_start(
                    wk[:, :, 128:D],
                    wk_d.ap()[:, 128:D]
                    .rearrange("(c p) e -> p c e", p=128)
                    .bitcast(F32R),
                ),
            ),
            "wv": lambda: nc.sync.dma_start(
                wv[:], wv_d.ap().rearrange("(c p) e -> p c e", p=128).bitcast(F32R)
            ),
            "wq": lambda: nc.sync.dma_start(
                wq[:], wq_d.ap().rearrange("(c p) e -> p c e", p=128).bitcast(F32R)
            ),
            "wp": lambda: nc.sync.dma_start(
                wp[:], wp_d.ap().rearrange("(c p) e -> p c e", p=128).bitcast(F32R)
            ),
        }

        def load_weights(*names):
            for n in names:
                fn = _wloads.pop(n, None)
                if fn is not None:
                    fn()

        pending = None  # deferred final-projection stage (oT, rT, b, sb)

        def flush_pending():
            nonlocal pending
            if pending is None:
                return
            oT, rT, b, sb = pending
            for j in range(4):
                pf = pbp.tile([128, 512], F32, tag="pbig", name="pf")
                for m in range(NC):
                    nc.tensor.matmul(
                        pf[:],
                        oT[:, m, j * 128 : (j + 1) * 128],
                        wp[:, m, :],
                        start=(m == 0),
                        stop=(m == NC - 1),
                    )
                res = resp.tile([128, 512], F32)
                if rT is None:  # final block: host normalizes
                    nc.vector.tensor_copy(res[:], pf[:])
                else:
                    nc.vector.tensor_scalar_mul(res[:], pf[:], rT[:, j : j + 1])
                r0 = sb * 512 + j * 128
                nc.sync.dma_start(out_d.ap()[b, r0 : r0 + 128, :], res[:])
            pending = None

        batch_seq = [b for _ in range(rep) for b in range(b_sz)]
        for it, b in enumerate(batch_seq):
            is_last_batch = it == len(batch_seq) - 1
            nt_b = kv_tiles[b]
            kv_groups = make_groups(nt_b)
            mo_t = mpool.tile([128, nt_b], F32)
            nc.sync.dma_start(mo_t[:], mo_d.ap()[b][:, :nt_b])

            # ---- stage P: load xT slices, project k^T and v ----
            xTq = xtqp.tile([128, NC, s_sz], F32R)
            xTk = xtkp.tile([128, NC, nt_b * 128], F32R)
            kT = ktp.tile([128, NC, nt_b * 128], F32R)
            v = vp.tile([128, nt_b, D], F32R)
            for gi, tiles in enumerate(kv_groups):
                last = gi == len(kv_groups) - 1
                n0 = tiles[0] * 128
                nw = len(tiles) * 128
                load_weights("wk0")
                for c in range(NC):
                    nc.sync.dma_start(
                        xTk[:, c, n0 : n0 + nw],
                        xkvt_d.ap()[b, c, :, n0 : n0 + nw].bitcast(F32R),
                    )
                load_weights("wk")
                if gi >= 1 or last:
                    load_weights("wq")
                if gi >= 2 or last:
                    load_weights("wp")
                # c-major across 4 parallel banks (po pool is idle in stage P)
                # so the PE issues 4 matmuls per arriving xTk chunk
                pks = [
                    pop.tile([128, 512], F32, tag="po", name=f"psk{m}")
                    for m in range(NC)
                ]
                for c in range(NC):
                    for m in range(NC):
                        nc.tensor.matmul(
                            pks[m][:, :nw],
                            wk[:, c, m * 128 : (m + 1) * 128],
                            xTk[:, c, n0 : n0 + nw],
                            start=(c == 0),
                            stop=(c == NC - 1),
                        )
                for m in range(NC):
                    nc.scalar.activation(
                        kT[:, m, n0 : n0 + nw],
                        pks[m][:, :nw],
                        AF.Identity,
                        bias=bk_t[:, m : m + 1],
                    )
                load_weights("wv")
                for t in tiles:
                    ps = psp.tile([128, 512], F32, tag="psmall", name="psv")
                    for c in range(NC):
                        nc.tensor.matmul(
                            ps[:],
                            xTk[:, c, t * 128 : (t + 1) * 128],
                            wv[:, c, :],
                            start=(c == 0),
                            stop=(c == NC - 1),
                        )
                    nc.vector.tensor_copy(v[:, t, :], ps[:])
                # stream the q-side xT in alongside (needed from stage A on)
                if gi < NC:
                    nc.sync.dma_start(
                        xTq[:, gi, :], xt_d.ap()[b, gi, :, :].bitcast(F32R)
                    )
            for c in range(len(kv_groups), NC):
                nc.sync.dma_start(xTq[:, c, :], xt_d.ap()[b, c, :, :].bitcast(F32R))

            # ---- stage A: per query-block attention ----
            for sb in range(NSB):
                qT = qtp.tile([128, NC, 512], F32R)
                for m in range(NC):
                    ps = psp.tile([128, 512], F32, tag="psmall", name="psq")
                    for c in range(NC):
                        nc.tensor.matmul(
                            ps[:],
                            wq[:, c, m * 128 : (m + 1) * 128],
                            xTq[:, c, sb * 512 : (sb + 1) * 512],
                            start=(c == 0),
                            stop=(c == NC - 1),
                        )
                    nc.scalar.activation(
                        qT[:, m, :], ps[:], AF.Identity, bias=bq_t[:, m : m + 1]
                    )

                flush_pending()

                po = [
                    pop.tile([128, 512], F32, tag="po", name=f"po{i}")
                    for i in range(NC)
                ]
                srun = srp.tile([128, 512], F32)
                srun_r = None

                def av_group(t, ptile):
                    for m in range(NC):
                        nc.tensor.matmul(
                            po[m][:],
                            v[:, t, m * 128 : (m + 1) * 128],
                            ptile[:],
                            start=(t == 0),
                            stop=(t == nt_b - 1),
                        )

                prev_av = None
                for t in range(nt_b):
                    ps = psp.tile([128, 512], F32, tag="psmall", name="pss")
                    for c in range(NC):
                        nc.tensor.matmul(
                            ps[:],
                            kT[:, c, t * 128 : (t + 1) * 128],
                            qT[:, c, :],
                            start=(c == 0),
                            stop=(c == NC - 1),
                        )
                    if prev_av is not None:
                        av_group(*prev_av)
                    ptile = ptp.tile([128, 512], F32R)
                    nc.scalar.activation(
                        ptile[:], ps[:], AF.Exp, bias=mo_t[:, t : t + 1]
                    )
                    if t < nt_b - 1:
                        if t == 0:
                            nc.vector.tensor_copy(srun[:], ptile[:].bitcast(F32))
                        else:
                            nc.vector.tensor_add(
                                srun[:], srun[:], ptile[:].bitcast(F32)
                            )
                    else:
                        # final step rounds to f32r so the partition-reduce
                        # matmul below runs at f32r speed
                        srun_r = srp.tile([128, 512], F32R, name="srun_r")
                        if t == 0:
                            nc.vector.tensor_copy(srun_r[:], ptile[:])
                        else:
                            nc.vector.tensor_add(
                                srun_r[:], srun[:], ptile[:].bitcast(F32)
                            )
                    prev_av = (t, ptile)
                av_group(*prev_av)
                # partition-reduce the running sum
                pd = pbp.tile([1, 512], F32, tag="pbig")
                nc.tensor.matmul(pd[:], ones[:], srun_r[:], start=True, stop=True)

                oT = otp.tile([128, NC, 512], F32R)
                for m in range(NC):
                    nc.vector.tensor_copy(oT[:, m, :], po[m][:])
                if is_last_batch and sb == NSB - 1:
                    # final block: ship the denominators; host divides
                    den = recp.tile([1, 512], F32)
                    nc.vector.tensor_copy(den[:], pd[:])
                    nc.sync.dma_start(dl_d.ap(), den[:])
                    rT = None
                else:
                    den = recp.tile([1, 512], F32)
                    nc.vector.tensor_copy(den[:], pd[:])
                    dscr = drp.tile([1, 512], F32)
                    nc.sync.dma_start(dscr[:], den[:])
                    denT = recp.tile([128, 4], F32)
                    nc.sync.dma_start(
                        denT[:], dscr[0, :].rearrange("(j p) -> p j", p=128)
                    )
                    rT = recp.tile([128, 4], F32)
                    nc.vector.reciprocal(rT[:], denT[:])

                pending = (oT, rT, b, sb)

        flush_pending()


@lru_cache(maxsize=4)
def _build(b_sz, s_sz, kv_tiles, rep=1):
    nc = bacc.Bacc("TRN2", target_bir_lowering=False, debug=False)
    _emit(nc, b_sz, s_sz, kv_tiles, rep=rep)
    nc.compile()
    return nc


def _prep_inputs(x, mask, Wq, bq, Wk, bk, Wv, bv, Wp, bp):
    """Host-side shard prep. Returns (in_maps, bp_eff, kv_tiles)."""
    b_sz, s_sz, _ = x.shape
    nc_ = D // 128
    x = np.asarray(x, dtype=np.float32)
    m = np.asarray(mask).reshape(b_sz, s_sz)
    counts = (m != 0).sum(axis=1)
    kv_tiles = tuple(max(1, int(-(-int(c) // 128))) for c in counts)
    s_kv = max(kv_tiles) * 128
    nt_kv = s_kv // 128
    # compact the key sequence: gather valid rows, pad with zeros + mask
    x_kv = np.zeros((b_sz, s_kv, D), dtype=np.float32)
    moff = np.full((b_sz, s_kv), np.float32(MASK_NEG), dtype=np.float32)
    for b in range(b_sz):
        idx = np.nonzero(m[b])[0]
        x_kv[b, : len(idx)] = x[b, idx]
        moff[b, : len(idx)] = 0.0
    moff = np.ascontiguousarray(moff.reshape(b_sz, nt_kv, 128).transpose(0, 2, 1))
    # feature-major (transposed) copies: [b, c, p, s]
    xt = np.ascontiguousarray(
        x.transpose(0, 2, 1).reshape(b_sz, nc_, 128, s_sz)
    )
    xkvt = np.ascontiguousarray(
        x_kv.transpose(0, 2, 1).reshape(b_sz, nc_, 128, s_kv)
    )

    sc = 1.0 / math.sqrt(D)
    in_maps = []
    for h in range(NCORES):
        wq_h = np.ascontiguousarray(np.asarray(Wq[h], dtype=np.float32) * sc)
        bq_h = (np.asarray(bq[h], dtype=np.float32) * sc).reshape(4, 128).T
        bk_h = np.asarray(bk[h], dtype=np.float32).reshape(4, 128).T
        in_maps.append(
            {
                "xt": xt,
                "xkvt": xkvt,
                "maskoff": moff,
                "wq": wq_h,
                "wk": np.ascontiguousarray(np.asarray(Wk[h], dtype=np.float32)),
                "wv": np.ascontiguousarray(np.asarray(Wv[h], dtype=np.float32)),
                "wp": np.ascontiguousarray(
                    np.asarray(Wp[h * D : (h + 1) * D, :], dtype=np.float32)
                ),
                "bq": np.ascontiguousarray(bq_h),
                "bk": np.ascontiguousarray(bk_h),
            }
        )
    bv64 = np.asarray(bv, dtype=np.float64)
    wp64 = np.asarray(Wp, dtype=np.float64)
    bp_eff = np.asarray(bp, dtype=np.float64).copy()
    for h in range(NCORES):
        bp_eff += bv64[h] @ wp64[h * D : (h + 1) * D, :]
    return in_maps, bp_eff.astype(np.float32), kv_tiles


def combine_results(results, bp_eff, b_sz, s_sz):
    """Sum per-head partials; the final query block arrives unnormalized with
    its denominators in 'denlast'."""
    acc = np.zeros((b_sz, s_sz, D), dtype=np.float64)
    for h in range(NCORES):
        o = np.asarray(results[h]["out"], dtype=np.float64)
        den = np.asarray(results[h]["denlast"], dtype=np.float64).reshape(512)
        acc[: b_sz - 1] += o[: b_sz - 1]
        acc[b_sz - 1, : s_sz - 512] += o[b_sz - 1, : s_sz - 512]
        acc[b_sz - 1, s_sz - 512 :] += o[b_sz - 1, s_sz - 512 :] / den[:, None]
    acc += bp_eff
    return acc.astype(np.float32)


def kernel(x, mask, Wq, bq, Wk, bk, Wv, bv, Wp, bp):
    x = np.asarray(x)
    b_sz, s_sz, _ = x.shape
    in_maps, bp_eff, kv_tiles = _prep_inputs(x, mask, Wq, bq, Wk, bk, Wv, bv, Wp, bp)
    nc = _build(b_sz, s_sz, kv_tiles)
    res = run_bass_kernel_spmd(nc, in_maps, list(range(NCORES)))
    return combine_results(res.results, bp_eff, b_sz, s_sz)
